# revision 10
# baseline (speedup 1.0000x reference)
"""DeltaNet chunked delta-rule kernel for Trainium2 (Bass/Tile), 8-core SPMD.

Full inputs: q,k,v [4,8,4096,128] fp32, beta [4,8,4096] fp32.
Sharding: 32 (b,h) pairs -> 4 per core across 8 cores (state S is per (b,h)).

Algorithm (identical to the CHUNK=32 reference for any chunk size; C=128):
  kh = l2norm(k), per chunk: T = (beta*kh) @ kh^T; P = -stril(T)
  inv = (I+P)(I+P2)(I+P4)  (truncated Neumann, exact through P^7)
  u = inv @ diag(beta) @ (v - kh@S); out = l2norm(q)@S + tril(qh kh^T)@u
  S += kh^T u

v6 design notes (trace-driven; ~2.2x over the 381us baseline of this series):
- Per-instruction fixed costs dominate at [128,128] tile sizes (DVE ~190ns,
  ACT ~300ns, PE MM ~56-80ns warm): minimize op COUNT on every engine, put
  adds on the PE (identity matmuls), merge drains into multi-slot ops across
  chunk PAIRS, and keep the PE stream dense so HAM stays at 2.4GHz (the old
  kernel ran its matmuls cold at 1.2GHz).
- ALL layout/scaling prep lives on the host: kT=l2norm(k)^T, kbT=(beta*kh)^T,
  qT=q^T are shipped pre-transposed+bf16 as one load-tile-shaped tensor (no
  PE/DMA transposes on device at all); kh, v packed as a second tensor; only
  beta is shipped for on-device use (invT row-scale). rq is applied to the
  OUTPUT on the host (both out terms are linear in qh's rows), so no q-norm,
  qh, or vb exist anywhere on device.
- Chain matmuls are slot-major in PSUM so the two I/PT1 accumulation terms
  and the stage-C R1 term run as single N=256 matmuls over the chunk pair.
- S is accumulated across all 32 chunks in a persistent PSUM bank
  (start=True only on the bank's very first matmul per repeat; later seqs'
  first writes land on cleared has_written bits and overwrite correctly);
  Sb is a per-iter 4-seq merged bf16 cast.
- invT is drained row-scaled by beta (invT' = diag(beta)@invT), pair-merged
  with a stride-0-broadcast in1 multiply.
"""
import numpy as np
import ml_dtypes

import concourse.bass as bass
import concourse.mybir as mybir
import concourse.tile as tile
from concourse import bacc
from concourse.bass_utils import run_bass_kernel_spmd
from concourse.masks import make_identity, make_lower_triangular, make_upper_triangular

B, H, L, D = 4, 8, 4096, 128
C = 128
NT = L // C
G = 4                 # chunks per load-group
NG = NT // G
NSEQ = (B * H) // 8   # sequences per core
FP = mybir.dt.float32
BF = mybir.dt.bfloat16
EPS = 1e-6
AF = mybir.ActivationFunctionType
ALU = mybir.AluOpType
BF_NP = ml_dtypes.bfloat16


def _emit_pair_pre(nc, work, psum, cst, bet_t, trsg, s, j):
    """Pre-scan work for chunks (2j, 2j+1) of one sequence: T/Tt/attnT and the
    Neumann chain -> invT' (beta-row-scaled), pair-merged drains."""
    identB, identB2 = cst["identB"], cst["identB2"]
    j2 = j % (G // 2)

    # ta: per chunk slot0 = T = kb@kh^T, slots1:3 = [Tt | attnT_raw] (one wide MM)
    ta_ps = psum.tile([C, 2, 3, C], FP, tag="tap", name="ta_ps", bufs=1)
    for jj in range(2):
        gi = 2 * j2 + jj
        kT = trsg[:, gi, 0, :]
        kbT = trsg[:, gi, 1, :]
        nc.tensor.matmul(ta_ps[:, jj, 0, :], kbT, kT)
        nc.tensor.matmul(ta_ps[:, jj, 1:3, :], kT, trsg[:, gi, 1:3, :])
    # ppa is SLOT-major in SBUF: [C, slot(P1,PT1,attnT), chunk, C]
    ppa = work.tile([C, 3, 2, C], BF, tag="ppa", name="ppa")
    nc.vector.tensor_tensor(out=ppa.rearrange("p s c f -> p c s f"),
                            in0=ta_ps, in1=cst["mPA3"], op=ALU.mult)

    # chain bank: SLOT-major [C, 2(slot), 2(chunk), C], 3-stage slot reuse
    ch_ps = psum.tile([C, 2, 2, C], FP, tag="chp", name="ch_ps", bufs=2)
    # stage A: P2 = P1@P1 (via PT1 stationary), PT2 = (P1@P1)^T
    for jj in range(2):
        P1, PT1 = ppa[:, 0, jj, :], ppa[:, 1, jj, :]
        nc.tensor.matmul(ch_ps[:, 0, jj, :], PT1, P1)
        nc.tensor.matmul(ch_ps[:, 1, jj, :], P1, PT1)
    PPa = work.tile([C, 2, 2, C], BF, tag="PPa", name="PPa")
    nc.scalar.copy(PPa, ch_ps)
    # stage B: slot0 = P4 (per chunk), slot1-pair = R1 = I + PT1 + PT2 + PT2@PT1
    # I and PT1 terms ride two wide N=256 matmuls over the pair. The P4
    # matmuls (implicit start=True, whole-bank has_written clear) MUST come
    # before the R1 accumulation group opens.
    for jj in range(2):
        nc.tensor.matmul(ch_ps[:, 0, jj, :], PPa[:, 1, jj, :], PPa[:, 0, jj, :])
    nc.tensor.matmul(ch_ps[:, 1, :, :], identB, identB2,
                     start=True, stop=False, skip_group_check=True)
    nc.tensor.matmul(ch_ps[:, 1, :, :], identB, ppa[:, 1, :, :],
                     start=False, stop=False, skip_group_check=True)
    for jj in range(2):
        PT1 = ppa[:, 1, jj, :]
        P2 = PPa[:, 0, jj, :]
        nc.tensor.matmul(ch_ps[:, 1, jj, :], P2, identB,
                         start=False, stop=False, skip_group_check=True)
        nc.tensor.matmul(ch_ps[:, 1, jj, :], P2, PT1,
                         start=False, stop=(jj == 1), skip_group_check=True)
    PPb = work.tile([C, 2, 2, C], BF, tag="PPb", name="PPb")
    nc.scalar.copy(PPb, ch_ps)
    # stage C: slot0-pair = V = (I + PT4) @ R1; R1 term rides one wide matmul
    nc.tensor.matmul(ch_ps[:, 0, :, :], identB, PPb[:, 1, :, :],
                     start=True, stop=False, skip_group_check=True)
    for jj in range(2):
        nc.tensor.matmul(ch_ps[:, 0, jj, :], PPb[:, 0, jj, :], PPb[:, 1, jj, :],
                         start=False, stop=(jj == 1), skip_group_check=True)
    invT2 = work.tile([C, 2, C], BF, tag="invT2", name="invT2")
    bpair = bet_t[:, 2 * j:2 * j + 2].unsqueeze(2).broadcast_to([C, 2, C])
    nc.vector.tensor_tensor(out=invT2, in0=ch_ps[:, 0, :, :],
                            in1=bpair, op=ALU.mult)
    return dict(ppa=ppa, invT2=invT2)


def _emit_scan(nc, work, psum, pre, S4, Sb4, kv, trsg, outg, s, i, first):
    """Serial per-chunk scan: z, y, u, o, S update."""
    jj = i % 2
    gi = i % G
    ppa, invT2 = pre["ppa"], pre["invT2"]
    kT = trsg[:, gi, 0, :]
    qT = trsg[:, gi, 2, :]
    attnT = ppa[:, 2, jj, :]
    sc = psum.tile([C, 2, D], FP, tag="scp", name="sc", bufs=3)
    nc.tensor.matmul(sc[:, 0, :], kT, Sb4[:, s, :])            # z = kh@Sb
    y = work.tile([C, D], BF, tag="y", name="y")
    nc.vector.tensor_tensor(out=y, in0=kv[:, 1, gi, :], in1=sc[:, 0, :],
                            op=ALU.subtract)                   # y = v - z
    nc.tensor.matmul(sc[:, 1, :], invT2[:, jj, :], y)          # u = inv b y
    nc.tensor.matmul(sc[:, 0, :], qT, Sb4[:, s, :], start=True, stop=False)
    u_bf = work.tile([C, D], BF, tag="u_bf", name="u_bf")
    nc.scalar.copy(u_bf, sc[:, 1, :])
    nc.tensor.matmul(sc[:, 0, :], attnT, u_bf, start=False, stop=True)
    nc.scalar.copy(outg[:, gi, :], sc[:, 0, :])
    nc.tensor.matmul(S4[:, s, :], kv[:, 0, gi, :], u_bf,
                     start=first, stop=(i == NT - 1), skip_group_check=True)


def build_nc(nseq=NSEQ, nt=NT, repeat=1):
    assert nt % (2 * G) == 0
    ng = nt // G
    nc = bacc.Bacc(None, target_bir_lowering=False)
    dram = {
        "tq": nc.dram_tensor("tq", [nseq, ng, D, G, 3, C], BF, kind="ExternalInput"),
        "kv": nc.dram_tensor("kv", [nseq, ng, C, 2, G, D], BF, kind="ExternalInput"),
        "bet": nc.dram_tensor("bet", [nseq, C, nt], FP, kind="ExternalInput"),
        "out": nc.dram_tensor("out", [nseq, ng, C, G, D], BF, kind="ExternalOutput"),
    }
    with tile.TileContext(nc) as tc:
        with (
            tc.tile_pool(name="consts", bufs=1) as consts,
            tc.tile_pool(name="persist", bufs=1) as persist,
            tc.tile_pool(name="grp", bufs=8) as grp,
            tc.tile_pool(name="work", bufs=6) as work,
            tc.tile_pool(name="psum", bufs=1, space="PSUM") as psum,
        ):
            identF = consts.tile([128, 128], FP, tag="identF", name="identF")
            identB = consts.tile([128, 128], BF, tag="identB", name="identB")
            identB2 = consts.tile([C, 2, C], BF, tag="identB2", name="identB2")
            mPA3 = consts.tile([C, 2, 3, C], FP, tag="mPA3", name="mPA3")
            make_identity(nc, identF)
            nc.vector.tensor_copy(identB, identF)
            for jj in range(2):
                nc.vector.tensor_copy(identB2[:, jj, :], identF)
                make_lower_triangular(nc, mPA3[:, jj, 0, :], val=-1.0, diag=False)
                make_upper_triangular(nc, mPA3[:, jj, 1, :], val=-1.0, diag=False)
                make_upper_triangular(nc, mPA3[:, jj, 2, :], val=1.0, diag=True)
            cst = dict(identB=identB, identB2=identB2, mPA3=mPA3)

            bet_t = []
            for s in range(nseq):
                bt = persist.tile([C, nt], FP, tag=f"bet{s}", name=f"bet{s}")
                nc.sync.dma_start(out=bt, in_=dram["bet"][s])
                bet_t.append(bt)
            Sb4 = persist.tile([D, nseq, D], BF, tag="Sb4", name="Sb4")
            S4 = psum.tile([D, nseq, D], FP, tag="S4", name="S4", bufs=1)

            for rep in range(repeat):
                nc.gpsimd.memset(Sb4, 0.0)
                for g in range(ng):
                    kv_t, trsg_t, outg_t = [], [], []
                    for s in range(nseq):
                        kt = grp.tile([C, 2, G, D], BF, tag="kv", name="kv_t")
                        nc.sync.dma_start(out=kt, in_=dram["kv"][s, g])
                        kv_t.append(kt)
                        tg = grp.tile([D, G, 3, C], BF, tag="trsg", name="trsg")
                        nc.sync.dma_start(out=tg, in_=dram["tq"][s, g])
                        trsg_t.append(tg)
                        outg_t.append(grp.tile([C, G, D], BF, tag="outg",
                                               name="outg"))
                    for j2 in range(G // 2):
                        j = g * (G // 2) + j2
                        pres = [_emit_pair_pre(nc, work, psum, cst, bet_t[s],
                                               trsg_t[s], s, j)
                                for s in range(nseq)]
                        for jj in range(2):
                            i = 2 * j + jj
                            for s in range(nseq):
                                _emit_scan(nc, work, psum, pres[s], S4, Sb4,
                                           kv_t[s], trsg_t[s], outg_t[s], s, i,
                                           first=(s == 0 and i == 0))
                            if i < nt - 1:
                                nc.vector.tensor_copy(Sb4, S4)
                    for s in range(nseq):
                        nc.sync.dma_start(out=dram["out"][s, g],
                                          in_=outg_t[s])
    nc.compile()
    return nc


_NC_CACHE = None


def _prep_inputs(q, k, v, beta):
    """Host-side: fp32 norms, transposes, bf16 packing."""
    nseq_all = B * H
    qf = np.ascontiguousarray(np.asarray(q, dtype=np.float32)).reshape(nseq_all, L, D)
    kf = np.ascontiguousarray(np.asarray(k, dtype=np.float32)).reshape(nseq_all, L, D)
    vf = np.ascontiguousarray(np.asarray(v, dtype=np.float32)).reshape(nseq_all, L, D)
    bf_ = np.ascontiguousarray(np.asarray(beta, dtype=np.float32)).reshape(nseq_all, L)
    rq = 1.0 / np.sqrt((qf.astype(np.float64) ** 2).sum(-1) + EPS)
    rq = rq.astype(np.float32)
    rk = 1.0 / np.sqrt((kf.astype(np.float64) ** 2).sum(-1) + EPS)
    rk = rk.astype(np.float32)
    kh = kf * rk[:, :, None]
    kb = kh * bf_[:, :, None]
    # tq: (kT, kbT, qT) -> [nseq, NG, D, G, 3, C]
    def tr(x):  # [ns, L, D] -> [ns, NG, D, G, C]
        return x.reshape(nseq_all, NG, G, C, D).transpose(0, 1, 4, 2, 3)
    tq = np.stack([tr(kh), tr(kb), tr(qf)], axis=4)  # [ns, NG, D, G, 3, C]
    tq = np.ascontiguousarray(tq.astype(BF_NP))
    # kv: (kh, v) -> [nseq, NG, C, 2, G, D]
    def pack(x):
        return x.reshape(nseq_all, NG, G, C, D).transpose(0, 1, 3, 2, 4)
    kv = np.stack([pack(kh), pack(vf)], axis=3)
    kv = np.ascontiguousarray(kv.astype(BF_NP))
    # bet: [nseq, C, NT] fp32
    bet = np.ascontiguousarray(
        bf_.reshape(nseq_all, NT, C).transpose(0, 2, 1).astype(np.float32))
    return dict(tq=tq, kv=kv, bet=bet), rq


def kernel(q, k, v, beta):
    global _NC_CACHE
    if _NC_CACHE is None:
        _NC_CACHE = build_nc()
    nc = _NC_CACHE
    full, rq = _prep_inputs(q, k, v, beta)
    in_maps = []
    for core in range(8):
        sl = slice(core * NSEQ, (core + 1) * NSEQ)
        in_maps.append({n: np.ascontiguousarray(a[sl]) for n, a in full.items()})
    res = run_bass_kernel_spmd(nc, in_maps, core_ids=list(range(8)))
    out = np.empty((B * H, L, D), dtype=np.float32)
    for core in range(8):
        ob = np.asarray(res.results[core]["out"], dtype=np.float32)
        # [NSEQ, NG, C, G, D] -> [NSEQ, NG, G, C, D] -> [NSEQ, L, D]
        sl = slice(core * NSEQ, (core + 1) * NSEQ)
        out[sl] = ob.transpose(0, 1, 3, 2, 4).reshape(NSEQ, L, D)
    out *= rq[:, :, None]
    return out.reshape(B, H, L, D)


# revision 11
# speedup vs baseline: 2.8559x; 2.8559x over previous
"""DeltaNet chunked delta-rule kernel for Trainium2 (Bass/Tile), 8-core SPMD.

Full inputs: q,k,v [4,8,4096,128] fp32, beta [4,8,4096] fp32.
Sharding: 32 (b,h) pairs -> 4 per core across 8 cores (state S is per (b,h)).

Algorithm (identical to the CHUNK=32 reference for any chunk size; C=128):
  kh = l2norm(k), per chunk: T = (beta*kh) @ kh^T; P = -stril(T)
  inv = (I+P)(I+P2)(I+P4)  (truncated Neumann, exact through P^7)
  u = inv @ diag(beta) @ (v - kh@S); out = l2norm(q)@S + tril(qh kh^T)@u
  S += kh^T u

v6 design notes (trace-driven; ~2.2x over the 381us baseline of this series):
- Per-instruction fixed costs dominate at [128,128] tile sizes (DVE ~190ns,
  ACT ~300ns, PE MM ~56-80ns warm): minimize op COUNT on every engine, put
  adds on the PE (identity matmuls), merge drains into multi-slot ops across
  chunk PAIRS, and keep the PE stream dense so HAM stays at 2.4GHz (the old
  kernel ran its matmuls cold at 1.2GHz).
- ALL layout/scaling prep lives on the host: kT=l2norm(k)^T, kbT=(beta*kh)^T,
  qT=q^T are shipped pre-transposed+bf16 as one load-tile-shaped tensor (no
  PE/DMA transposes on device at all); kh, v packed as a second tensor; only
  beta is shipped for on-device use (invT row-scale). rq is applied to the
  OUTPUT on the host (both out terms are linear in qh's rows), so no q-norm,
  qh, or vb exist anywhere on device.
- Chain matmuls are slot-major in PSUM so the two I/PT1 accumulation terms
  and the stage-C R1 term run as single N=256 matmuls over the chunk pair.
- S is accumulated across all 32 chunks in a persistent PSUM bank
  (start=True only on the bank's very first matmul per repeat; later seqs'
  first writes land on cleared has_written bits and overwrite correctly);
  Sb is a per-iter 4-seq merged bf16 cast.
- invT is drained row-scaled by beta (invT' = diag(beta)@invT), pair-merged
  with a stride-0-broadcast in1 multiply.
"""
import numpy as np
import ml_dtypes

import concourse.bass as bass
import concourse.mybir as mybir
import concourse.tile as tile
from concourse import bacc
from concourse.bass_utils import run_bass_kernel_spmd
from concourse.masks import make_identity, make_lower_triangular, make_upper_triangular

B, H, L, D = 4, 8, 4096, 128
C = 128
NT = L // C
G = 4                 # chunks per load-group
NG = NT // G
NSEQ = (B * H) // 8   # sequences per core
FP = mybir.dt.float32
BF = mybir.dt.bfloat16
EPS = 1e-6
AF = mybir.ActivationFunctionType
ALU = mybir.AluOpType
BF_NP = ml_dtypes.bfloat16


def _emit_pair_pre(nc, work, psum, cst, bet_t, trsg, s, j):
    """Pre-scan work for chunks (2j, 2j+1) of one sequence: T/Tt/attnT and the
    Neumann chain -> invT' (beta-row-scaled), pair-merged drains."""
    identB, identB2 = cst["identB"], cst["identB2"]
    j2 = j % (G // 2)

    # ta: per chunk slot0 = T = kb@kh^T, slots1:3 = [Tt | attnT_raw] (one wide MM)
    ta_ps = psum.tile([C, 2, 3, C], FP, tag="tap", name="ta_ps", bufs=1)
    for jj in range(2):
        gi = 2 * j2 + jj
        kT = trsg[:, gi, 0, :]
        kbT = trsg[:, gi, 1, :]
        nc.tensor.matmul(ta_ps[:, jj, 0, :], kbT, kT)
        nc.tensor.matmul(ta_ps[:, jj, 1:3, :], kT, trsg[:, gi, 1:3, :])
    # ppa is SLOT-major in SBUF: [C, slot(P1,PT1,attnT), chunk, C]
    ppa = work.tile([C, 3, 2, C], BF, tag="ppa", name="ppa")
    nc.vector.tensor_tensor(out=ppa.rearrange("p s c f -> p c s f"),
                            in0=ta_ps, in1=cst["mPA3"], op=ALU.mult)

    # chain bank: SLOT-major [C, 2(slot), 2(chunk), C], 3-stage slot reuse
    ch_ps = psum.tile([C, 2, 2, C], FP, tag="chp", name="ch_ps", bufs=3)
    # stage A: P2 = P1@P1 (via PT1 stationary), PT2 = (P1@P1)^T
    for jj in range(2):
        P1, PT1 = ppa[:, 0, jj, :], ppa[:, 1, jj, :]
        nc.tensor.matmul(ch_ps[:, 0, jj, :], PT1, P1)
        nc.tensor.matmul(ch_ps[:, 1, jj, :], P1, PT1)
    PPa = work.tile([C, 2, 2, C], BF, tag="PPa", name="PPa")
    nc.scalar.copy(PPa, ch_ps)
    # stage B: slot0 = P4 (per chunk), slot1-pair = R1 = I + PT1 + PT2 + PT2@PT1
    # I and PT1 terms ride two wide N=256 matmuls over the pair. The P4
    # matmuls (implicit start=True, whole-bank has_written clear) MUST come
    # before the R1 accumulation group opens.
    for jj in range(2):
        nc.tensor.matmul(ch_ps[:, 0, jj, :], PPa[:, 1, jj, :], PPa[:, 0, jj, :])
    nc.tensor.matmul(ch_ps[:, 1, :, :], identB, identB2,
                     start=True, stop=False, skip_group_check=True)
    nc.tensor.matmul(ch_ps[:, 1, :, :], identB, ppa[:, 1, :, :],
                     start=False, stop=False, skip_group_check=True)
    for jj in range(2):
        PT1 = ppa[:, 1, jj, :]
        P2 = PPa[:, 0, jj, :]
        nc.tensor.matmul(ch_ps[:, 1, jj, :], P2, identB,
                         start=False, stop=False, skip_group_check=True)
        nc.tensor.matmul(ch_ps[:, 1, jj, :], P2, PT1,
                         start=False, stop=(jj == 1), skip_group_check=True)
    PPb = work.tile([C, 2, 2, C], BF, tag="PPb", name="PPb")
    nc.scalar.copy(PPb, ch_ps)
    # stage C: slot0-pair = V = (I + PT4) @ R1; R1 term rides one wide matmul
    nc.tensor.matmul(ch_ps[:, 0, :, :], identB, PPb[:, 1, :, :],
                     start=True, stop=False, skip_group_check=True)
    for jj in range(2):
        nc.tensor.matmul(ch_ps[:, 0, jj, :], PPb[:, 0, jj, :], PPb[:, 1, jj, :],
                         start=False, stop=(jj == 1), skip_group_check=True)
    invT2 = work.tile([C, 2, C], BF, tag="invT2", name="invT2")
    bpair = bet_t[:, 2 * j:2 * j + 2].unsqueeze(2).broadcast_to([C, 2, C])
    nc.vector.tensor_tensor(out=invT2, in0=ch_ps[:, 0, :, :],
                            in1=bpair, op=ALU.mult)
    return dict(ppa=ppa, invT2=invT2)


def _emit_scan(nc, work, psum, pre, S4, Sb4, kv, trsg, outg, s, i, first):
    """Serial per-chunk scan: z, y, u, o, S update."""
    jj = i % 2
    gi = i % G
    ppa, invT2 = pre["ppa"], pre["invT2"]
    kT = trsg[:, gi, 0, :]
    qT = trsg[:, gi, 2, :]
    attnT = ppa[:, 2, jj, :]
    sc = psum.tile([C, 2, D], FP, tag="scp", name="sc", bufs=2)
    nc.tensor.matmul(sc[:, 0, :], kT, Sb4[:, s, :])            # z = kh@Sb
    y = work.tile([C, D], BF, tag="y", name="y")
    nc.vector.tensor_tensor(out=y, in0=kv[:, 1, gi, :], in1=sc[:, 0, :],
                            op=ALU.subtract)                   # y = v - z
    nc.tensor.matmul(sc[:, 1, :], invT2[:, jj, :], y)          # u = inv b y
    nc.tensor.matmul(sc[:, 0, :], qT, Sb4[:, s, :], start=True, stop=False)
    u_bf = work.tile([C, D], BF, tag="u_bf", name="u_bf")
    nc.scalar.copy(u_bf, sc[:, 1, :])
    nc.tensor.matmul(sc[:, 0, :], attnT, u_bf, start=False, stop=True)
    nc.scalar.copy(outg[:, gi, :], sc[:, 0, :])
    nc.tensor.matmul(S4[:, s, :], kv[:, 0, gi, :], u_bf,
                     start=first, stop=(i == NT - 1), skip_group_check=True)


def build_nc(nseq=NSEQ, nt=NT, repeat=1):
    assert nt % (2 * G) == 0
    ng = nt // G
    nc = bacc.Bacc(None, target_bir_lowering=False)
    dram = {
        "tq": nc.dram_tensor("tq", [nseq, ng, D, G, 3, C], BF, kind="ExternalInput"),
        "kv": nc.dram_tensor("kv", [nseq, ng, C, 2, G, D], BF, kind="ExternalInput"),
        "bet": nc.dram_tensor("bet", [nseq, C, nt], FP, kind="ExternalInput"),
        "out": nc.dram_tensor("out", [nseq, ng, C, G, D], BF, kind="ExternalOutput"),
    }
    with tile.TileContext(nc) as tc:
        with (
            tc.tile_pool(name="consts", bufs=1) as consts,
            tc.tile_pool(name="persist", bufs=1) as persist,
            tc.tile_pool(name="grp", bufs=8) as grp,
            tc.tile_pool(name="work", bufs=8) as work,
            tc.tile_pool(name="psum", bufs=1, space="PSUM") as psum,
        ):
            identF = consts.tile([128, 128], FP, tag="identF", name="identF")
            identB = consts.tile([128, 128], BF, tag="identB", name="identB")
            identB2 = consts.tile([C, 2, C], BF, tag="identB2", name="identB2")
            mPA3 = consts.tile([C, 2, 3, C], FP, tag="mPA3", name="mPA3")
            make_identity(nc, identF)
            nc.vector.tensor_copy(identB, identF)
            for jj in range(2):
                nc.vector.tensor_copy(identB2[:, jj, :], identF)
                make_lower_triangular(nc, mPA3[:, jj, 0, :], val=-1.0, diag=False)
                make_upper_triangular(nc, mPA3[:, jj, 1, :], val=-1.0, diag=False)
                make_upper_triangular(nc, mPA3[:, jj, 2, :], val=1.0, diag=True)
            cst = dict(identB=identB, identB2=identB2, mPA3=mPA3)

            bet_t = []
            for s in range(nseq):
                bt = persist.tile([C, nt], FP, tag=f"bet{s}", name=f"bet{s}")
                nc.sync.dma_start(out=bt, in_=dram["bet"][s])
                bet_t.append(bt)
            Sb4 = persist.tile([D, nseq, D], BF, tag="Sb4", name="Sb4")
            S4 = psum.tile([D, nseq, D], FP, tag="S4", name="S4", bufs=1)

            for rep in range(repeat):
                nc.gpsimd.memset(Sb4, 0.0)
                for g in range(ng):
                    kv_t, trsg_t, outg_t = [], [], []
                    for s in range(nseq):
                        kt = grp.tile([C, 2, G, D], BF, tag="kv", name="kv_t")
                        nc.sync.dma_start(out=kt, in_=dram["kv"][s, g])
                        kv_t.append(kt)
                        tg = grp.tile([D, G, 3, C], BF, tag="trsg", name="trsg")
                        nc.sync.dma_start(out=tg, in_=dram["tq"][s, g])
                        trsg_t.append(tg)
                        outg_t.append(grp.tile([C, G, D], BF, tag="outg",
                                               name="outg"))
                    for j2 in range(G // 2):
                        j = g * (G // 2) + j2
                        pres = [_emit_pair_pre(nc, work, psum, cst, bet_t[s],
                                               trsg_t[s], s, j)
                                for s in range(nseq)]
                        for jj in range(2):
                            i = 2 * j + jj
                            for s in range(nseq):
                                _emit_scan(nc, work, psum, pres[s], S4, Sb4,
                                           kv_t[s], trsg_t[s], outg_t[s], s, i,
                                           first=(s == 0 and i == 0))
                            if i < nt - 1:
                                nc.vector.tensor_copy(Sb4, S4)
                    for s in range(nseq):
                        nc.sync.dma_start(out=dram["out"][s, g],
                                          in_=outg_t[s])
    nc.compile()
    return nc


_NC_CACHE = None


def _prep_inputs(q, k, v, beta):
    """Host-side: fp32 norms, transposes, bf16 packing."""
    nseq_all = B * H
    qf = np.ascontiguousarray(np.asarray(q, dtype=np.float32)).reshape(nseq_all, L, D)
    kf = np.ascontiguousarray(np.asarray(k, dtype=np.float32)).reshape(nseq_all, L, D)
    vf = np.ascontiguousarray(np.asarray(v, dtype=np.float32)).reshape(nseq_all, L, D)
    bf_ = np.ascontiguousarray(np.asarray(beta, dtype=np.float32)).reshape(nseq_all, L)
    rq = 1.0 / np.sqrt((qf.astype(np.float64) ** 2).sum(-1) + EPS)
    rq = rq.astype(np.float32)
    rk = 1.0 / np.sqrt((kf.astype(np.float64) ** 2).sum(-1) + EPS)
    rk = rk.astype(np.float32)
    kh = kf * rk[:, :, None]
    kb = kh * bf_[:, :, None]
    # tq: (kT, kbT, qT) -> [nseq, NG, D, G, 3, C]
    def tr(x):  # [ns, L, D] -> [ns, NG, D, G, C]
        return x.reshape(nseq_all, NG, G, C, D).transpose(0, 1, 4, 2, 3)
    tq = np.stack([tr(kh), tr(kb), tr(qf)], axis=4)  # [ns, NG, D, G, 3, C]
    tq = np.ascontiguousarray(tq.astype(BF_NP))
    # kv: (kh, v) -> [nseq, NG, C, 2, G, D]
    def pack(x):
        return x.reshape(nseq_all, NG, G, C, D).transpose(0, 1, 3, 2, 4)
    kv = np.stack([pack(kh), pack(vf)], axis=3)
    kv = np.ascontiguousarray(kv.astype(BF_NP))
    # bet: [nseq, C, NT] fp32
    bet = np.ascontiguousarray(
        bf_.reshape(nseq_all, NT, C).transpose(0, 2, 1).astype(np.float32))
    return dict(tq=tq, kv=kv, bet=bet), rq


def kernel(q, k, v, beta):
    global _NC_CACHE
    if _NC_CACHE is None:
        _NC_CACHE = build_nc()
    nc = _NC_CACHE
    full, rq = _prep_inputs(q, k, v, beta)
    in_maps = []
    for core in range(8):
        sl = slice(core * NSEQ, (core + 1) * NSEQ)
        in_maps.append({n: np.ascontiguousarray(a[sl]) for n, a in full.items()})
    res = run_bass_kernel_spmd(nc, in_maps, core_ids=list(range(8)))
    out = np.empty((B * H, L, D), dtype=np.float32)
    for core in range(8):
        ob = np.asarray(res.results[core]["out"], dtype=np.float32)
        # [NSEQ, NG, C, G, D] -> [NSEQ, NG, G, C, D] -> [NSEQ, L, D]
        sl = slice(core * NSEQ, (core + 1) * NSEQ)
        out[sl] = ob.transpose(0, 1, 3, 2, 4).reshape(NSEQ, L, D)
    out *= rq[:, :, None]
    return out.reshape(B, H, L, D)
